# revision 1
# baseline (speedup 1.0000x reference)
"""Distributed contrastive loss kernel for 8 Trainium2 NeuronCores.

loss = mean_i( logsumexp_j(f1n_i . f2n_j / T) - (f1n_i . f2n_i) / T )
with f1n/f2n the L2-row-normalized feature matrices, N=16384, D=512.

Sharding (classic distributed contrastive loss):
- f1 rows sharded 8 ways (2048/core); f2 rows likewise for prep.
- Each core normalizes+transposes only its own f2 shard (inv2 folded into a
  PE "transpose" against diag(inv2)); shards are AllGathered as bf16 [512,2048]
  blocks so every core holds the full transposed f2n.
- Main loop: per (m-tile, rank-block) unit, 16 bf16 matmuls accumulate a
  [128, 2048] f32 logits tile in PSUM, then ONE ScalarE Exp with per-partition
  scale = inv1/T and fused accum_out produces the row-wise exp-sums. The
  16384^2 logits never leave PSUM.
- diag from an elementwise dot of the local shards; logsumexp via Ln of the
  accumulated row sums; per-core partial reduced to a scalar with a ones-
  matmul. Host sums 8 partials / N.
"""

import os
from contextlib import ExitStack
from functools import lru_cache

import numpy as np

import concourse.bass as bass
import concourse.mybir as mybir
import concourse.tile as tile
from concourse.bass_utils import run_bass_kernel_spmd
from concourse.masks import make_identity

# Problem shape (hardcoded per contest rules).
N = 16384
D = 512
N_CORES = 8
M_LOCAL = N // N_CORES  # 2048 rows per core
TEMP = 0.07

P = 128                 # SBUF partitions
DC = D // P             # 4 contraction chunks
MT = M_LOCAL // P       # 16 m-tiles (and f2-shard tiles) per core
NCH = 512               # matmul moving-operand free dim (one PSUM bank)
EXPW = 2048             # exp tile width = one rank block of columns
F32 = mybir.dt.float32
BF16 = mybir.dt.bfloat16
AF = mybir.ActivationFunctionType
ALU = mybir.AluOpType

# Module-level stash for the last run's profile (read by test.py).
LAST_EXEC_TIME_NS = None


def _install_ntff_hook():
    """Provide antenv.axon_hooks (missing from this image) so that
    run_bass_kernel_spmd(trace=True) can capture NTFF profiles via the
    axon PJRT .so. Mirrors trn_agent_boot.trn_boot._ntff_profile_via_ctypes."""
    import contextlib
    import ctypes
    import sys
    import types

    try:
        import antenv.axon_hooks  # noqa: F401

        return
    except ImportError:
        pass

    so_path = "/opt/axon/libaxon_pjrt.so"
    hook = None
    try:
        lib = ctypes.CDLL(so_path)
        if hasattr(lib, "axon_start_nrt_profile"):
            lib.axon_start_nrt_profile.argtypes = [
                ctypes.POINTER(ctypes.c_int64),
                ctypes.c_size_t,
            ]
            lib.axon_start_nrt_profile.restype = ctypes.c_int64
            lib.axon_stop_nrt_profile.argtypes = [ctypes.c_char_p]
            lib.axon_stop_nrt_profile.restype = ctypes.c_int64

            @contextlib.contextmanager
            def _hook(output_dir, device_ids):
                import jax

                jax.devices()
                if device_ids:
                    ids = (ctypes.c_int64 * len(device_ids))(*device_ids)
                    rc = lib.axon_start_nrt_profile(ids, len(device_ids))
                else:
                    rc = lib.axon_start_nrt_profile(None, 0)
                if rc != 0:
                    raise RuntimeError(f"axon_start_nrt_profile rc={rc}")
                try:
                    yield
                finally:
                    n = lib.axon_stop_nrt_profile(str(output_dir).encode())
                    print(f"profile: {n} file(s) written to {output_dir}", file=sys.stderr)

            hook = _hook
    except OSError:
        pass

    import antenv

    mod = types.ModuleType("antenv.axon_hooks")
    _state = {"hook": hook}
    mod.set_axon_ntff_profile_hook = lambda h: _state.__setitem__("hook", h)
    mod.get_axon_ntff_profile_hook = lambda: _state["hook"]
    sys.modules["antenv.axon_hooks"] = mod
    antenv.axon_hooks = mod

    # Artifact upload needs external storage creds; make it a no-op.
    import concourse.bass_utils as _bu

    _bu.upload_artifacts = lambda tmpdir: f"local:{tmpdir}"


def _build_bass():
    nc = bass.Bass(num_devices=N_CORES, debug=False)

    f1s = nc.dram_tensor("f1s", [M_LOCAL, D], F32, kind="ExternalInput")
    f2o = nc.dram_tensor("f2o", [M_LOCAL, D], F32, kind="ExternalInput")
    out = nc.dram_tensor("out", [1, 1], F32, kind="ExternalOutput")

    inv_temp = 1.0 / TEMP
    NG = 4                  # AllGather chunks
    GT = MT // NG           # 4 f2-shard tiles per chunk
    GW = GT * P             # 512 columns of f2nT per chunk

    with tile.TileContext(nc) as tc, ExitStack() as ctx:
        consts = ctx.enter_context(tc.tile_pool(name="consts", bufs=1))
        resident = ctx.enter_context(tc.tile_pool(name="resident", bufs=1))
        loads = ctx.enter_context(tc.tile_pool(name="loads", bufs=4))
        work = ctx.enter_context(tc.tile_pool(name="work", bufs=3))
        stats = ctx.enter_context(tc.tile_pool(name="stats", bufs=4))
        rhsp = ctx.enter_context(tc.tile_pool(name="rhsp", bufs=24))
        psum = ctx.enter_context(tc.tile_pool(name="psum", bufs=2, space="PSUM"))
        dram = ctx.enter_context(tc.tile_pool(name="dram", bufs=1, space="DRAM"))

        identity = consts.tile([P, P], BF16)
        make_identity(nc, identity)
        ones_col = consts.tile([P, 1], F32)
        nc.vector.memset(ones_col, 1.0)

        # Residents.
        f1nT = [resident.tile([P, M_LOCAL], BF16, name=f"f1nT{c}") for c in range(DC)]
        f2nTloc = [
            resident.tile([P, M_LOCAL], BF16, name=f"f2nTloc{c}") for c in range(DC)
        ]
        xb2s = [
            resident.tile([P, D], BF16, name=f"xb2_{t}") for t in range(MT)
        ]
        ss1g = resident.tile([P, MT], F32, name="ss1g")
        ss2g = resident.tile([P, MT], F32, name="ss2g")
        diag_raw = resident.tile([P, MT], F32, name="diag_raw")
        diag = resident.tile([P, MT], F32, name="diag")
        inv1g = resident.tile([P, MT], F32, name="inv1g")
        inv2g = resident.tile([P, MT], F32, name="inv2g")
        scale1g = resident.tile([P, MT], F32, name="scale1g")
        rowsums = [
            resident.tile([P, N_CORES], F32, name=f"rs{mt}") for mt in range(MT)
        ]
        losses = resident.tile([P, MT], F32, name="losses")

        # AllGather bounce buffers, one pair per chunk.
        agin = [
            dram.tile([DC * P, GW], BF16, name=f"agin{g}") for g in range(NG)
        ]
        agout = [
            dram.tile([N_CORES * DC * P, GW], BF16, name=f"agout{g}",
                      addr_space="Shared")
            for g in range(NG)
        ]

        def sumsq_col(x, col, tag):
            """sum(x^2) per row of [P, D] f32 tile -> col ([P,1] slice). DVE only."""
            st = stats.tile([P, nc.vector.BN_STATS_DIM], F32, tag="bst", name=f"st{tag}")
            nc.vector.bn_stats(st, x)
            mv = stats.tile([P, nc.vector.BN_AGGR_DIM], F32, tag="bmv", name=f"mv{tag}")
            nc.vector.bn_aggr(mv, st)
            m2 = stats.tile([P, 1], F32, tag="m2", name=f"m2{tag}")
            nc.vector.tensor_tensor(m2, mv[:, 0:1], mv[:, 0:1], ALU.mult)
            s2 = stats.tile([P, 1], F32, tag="s2", name=f"s2{tag}")
            nc.vector.tensor_tensor(s2, mv[:, 1:2], m2, ALU.add)
            nc.vector.tensor_scalar_mul(col, s2, float(D))

        # ---- Phase 1+2, chunked: per group of 4 shard tiles:
        #      f1 prep + stats -> quarter rsqrt -> f2 diag(inv2)-transpose ->
        #      chunked AllGather (overlaps with the main loop below).
        def prep_group(g):
            for t in range(g * GT, (g + 1) * GT):
                x1 = loads.tile([P, D], F32, tag="x", name="x1")
                nc.sync.dma_start(out=x1, in_=f1s[t * P : (t + 1) * P, :])
                x2 = loads.tile([P, D], F32, tag="x", name="x2")
                nc.sync.dma_start(out=x2, in_=f2o[t * P : (t + 1) * P, :])

                xb1 = work.tile([P, D], BF16, tag="xb1", name="xb1")
                nc.vector.tensor_copy(xb1, x1)
                nc.vector.tensor_copy(xb2s[t], x2)

                sumsq_col(x1, ss1g[:, t : t + 1], "1")
                sumsq_col(x2, ss2g[:, t : t + 1], "2")

                prod = work.tile([P, D], F32, tag="prod", bufs=2, name="prod")
                nc.vector.tensor_tensor(prod, x1, x2, ALU.mult)
                nc.vector.reduce_sum(
                    diag_raw[:, t : t + 1], prod, axis=mybir.AxisListType.X
                )

                for c in range(DC):
                    tp = psum.tile([P, P], BF16, tag="ps", name="tp1")
                    nc.tensor.transpose(tp, xb1[:, c * P : (c + 1) * P], identity)
                    nc.vector.tensor_copy(f1nT[c][:, t * P : (t + 1) * P], tp)

            # Quarter-batched rsqrt via exp(-0.5*ln(ss)) — exp/ln table set.
            gsl = slice(g * GT, (g + 1) * GT)
            ln2 = stats.tile([P, GT], F32, tag="ln", name="ln2")
            nc.scalar.activation(ln2, ss2g[:, gsl], AF.Ln)
            nc.scalar.activation(inv2g[:, gsl], ln2, AF.Exp, scale=-0.5)

            for t in range(g * GT, (g + 1) * GT):
                dgm = work.tile([P, P], BF16, tag="dgm", bufs=2, name="dgm")
                nc.vector.tensor_scalar_mul(dgm, identity, inv2g[:, t : t + 1])
                for c in range(DC):
                    tp2 = psum.tile([P, P], F32, tag="ps", name="tp2")
                    nc.tensor.matmul(
                        tp2, lhsT=xb2s[t][:, c * P : (c + 1) * P], rhs=dgm,
                        start=True, stop=True,
                    )
                    nc.vector.tensor_copy(
                        f2nTloc[c][:, t * P : (t + 1) * P], tp2
                    )

            for c in range(DC):
                nc.sync.dma_start(
                    out=agin[g][c * P : (c + 1) * P, :],
                    in_=f2nTloc[c][:, g * GW : (g + 1) * GW],
                )
            nc.gpsimd.collective_compute(
                "AllGather",
                ALU.bypass,
                replica_groups=[list(range(N_CORES))],
                ins=[agin[g][:, :].opt()],
                outs=[agout[g][:, :].opt()],
            )

        def finish_scales(g):
            # inv1 / scale1g / diag for this quarter of m-tiles.
            gsl = slice(g * GT, (g + 1) * GT)
            ln1 = stats.tile([P, GT], F32, tag="ln1", name="ln1")
            nc.scalar.activation(ln1, ss1g[:, gsl], AF.Ln)
            nc.scalar.activation(inv1g[:, gsl], ln1, AF.Exp, scale=-0.5)
            nc.vector.tensor_scalar_mul(
                scale1g[:, gsl], inv1g[:, gsl], inv_temp
            )
            # diag = diag_raw * inv1 * inv2 (row-paired).
            nc.vector.tensor_tensor(
                diag[:, gsl], diag_raw[:, gsl], inv1g[:, gsl], ALU.mult
            )
            nc.vector.tensor_tensor(
                diag[:, gsl], diag[:, gsl], inv2g[:, gsl], ALU.mult
            )

        # ---- Phase 3: fused logits -> exp(scale=inv1/T) -> row-sums ---------
        # Unit (g, h, mt): [128, 2048] PSUM tile = chunk g columns of ranks
        # 4h..4h+3 (column order within logsumexp is irrelevant).
        def main_group(g):
            for h in range(2):
                rhs_t = {}
                for j in range(4):
                    r = 4 * h + j
                    for c in range(DC):
                        rt = rhsp.tile([P, GW], BF16, tag="rhs", name=f"rhs{c}_{j}")
                        nc.sync.dma_start(
                            out=rt,
                            in_=agout[g][r * D + c * P : r * D + (c + 1) * P, :],
                        )
                        rhs_t[(c, j)] = rt
                for mt in range(MT):
                    ps = psum.tile([P, EXPW], F32, tag="ps", name="ps")
                    for c in range(DC):
                        for j in range(4):
                            nc.tensor.matmul(
                                ps[:, j * GW : (j + 1) * GW],
                                lhsT=f1nT[c][:, mt * P : (mt + 1) * P],
                                rhs=rhs_t[(c, j)],
                                start=(c == 0),
                                stop=(c == DC - 1),
                            )
                    ex = work.tile([P, EXPW], BF16, tag="ex", bufs=2, name="ex")
                    nc.scalar.activation(
                        ex,
                        ps,
                        AF.Exp,
                        scale=scale1g[:, mt : mt + 1],
                        accum_out=rowsums[mt][:, 2 * g + h : 2 * g + h + 1],
                    )

        # Prep groups fire their AllGather as soon as their quarter is ready;
        # the AGs + later prep hide under the first main groups' compute.
        for g in range(NG):
            prep_group(g)
            finish_scales(g)
        for g in range(NG):
            main_group(g)

        # ---- Phase 4: logsumexp, subtract diag, reduce -----------------------
        for mt in range(MT):
            s = stats.tile([P, 1], F32, tag="s", name="s")
            nc.vector.reduce_sum(s, rowsums[mt], axis=mybir.AxisListType.X)
            lse = stats.tile([P, 1], F32, tag="lse", name="lse")
            nc.scalar.activation(lse, s, AF.Ln)
            # losses[:, mt] = lse - diag/T = (diag * -1/T) + lse
            nc.vector.scalar_tensor_tensor(
                out=losses[:, mt : mt + 1],
                in0=diag[:, mt : mt + 1],
                scalar=-inv_temp,
                in1=lse,
                op0=ALU.mult,
                op1=ALU.add,
            )

        loss_col = stats.tile([P, 1], F32, tag="lc", name="loss_col")
        nc.vector.reduce_sum(loss_col, losses, axis=mybir.AxisListType.X)
        fin = psum.tile([1, 1], F32, tag="ps", name="fin")
        nc.tensor.matmul(fin, lhsT=loss_col, rhs=ones_col, start=True, stop=True)
        res = stats.tile([1, 1], F32, tag="res", name="res")
        nc.any.tensor_copy(res, fin)
        nc.sync.dma_start(out=out[:, :], in_=res)

    return nc


_WAIT_EXEMPT = ("InstCall",)


def _legalize_sync_waits(nc, limit=1):
    """Walrus codegen rejects instructions carrying more than ~1 embedded
    semaphore wait ("Too many sync wait commands"). Move excess waits onto
    injected same-engine NoOps (one wait each) ahead of the instruction —
    semantically identical (the engine blocks on the NoOps first)."""
    n_split = 0
    for b in nc.m.functions[0].blocks:
        insts = b.instructions
        out = []
        changed = False
        for ins in insts:
            si = ins.sync_info
            tname = type(ins).__name__
            if (
                si is not None
                and len(si.on_wait) > limit
                and tname not in _WAIT_EXEMPT
            ):
                waits = list(si.on_wait)
                keep, excess = waits[:limit], waits[limit:]
                for j, w in enumerate(excess):
                    noop = mybir.InstNoOp(name=f"{ins.name}-ws{j}", ins=[], outs=[])
                    noop.engine = ins.engine
                    noop.sync_info = mybir.SyncInfo(on_wait=[w], on_update=[])
                    out.append(noop)
                ins.sync_info = mybir.SyncInfo(
                    on_wait=keep, on_update=list(si.on_update)
                )
                n_split += 1
                changed = True
            out.append(ins)
        if changed:
            b.instructions = out
    return n_split


def _maybe_patch_ldw_opt():
    """KERNEL_LDW_OPT=1 flips walrus --enable-ldw-opt to true (FWL weight
    loads); A/B experiment, correctness-checked by the rel-err gate."""
    if not int(os.environ.get("KERNEL_LDW_OPT", "0")):
        return
    import concourse.bass_utils as bu

    if getattr(bu.run_command, "_ldw_patched", False):
        return
    orig = bu.run_command

    def run2(cmd, **kw):
        cmd = [
            "--enable-ldw-opt=true" if c == "--enable-ldw-opt=false" else c
            for c in cmd
        ]
        return orig(cmd, **kw)

    run2._ldw_patched = True
    bu.run_command = run2


@lru_cache(maxsize=1)
def _get_nc():
    _maybe_patch_ldw_opt()
    nc = _build_bass()
    _legalize_sync_waits(nc)
    return nc


def kernel(features1, features2):
    global LAST_EXEC_TIME_NS
    f1 = np.ascontiguousarray(np.asarray(features1, dtype=np.float32))
    f2 = np.ascontiguousarray(np.asarray(features2, dtype=np.float32))
    assert f1.shape == (N, D) and f2.shape == (N, D)

    in_maps = []
    for i in range(N_CORES):
        sl = slice(i * M_LOCAL, (i + 1) * M_LOCAL)
        in_maps.append(
            {
                "f1s": np.ascontiguousarray(f1[sl]),
                "f2o": np.ascontiguousarray(f2[sl]),
            }
        )

    nc = _get_nc()
    trace = bool(int(os.environ.get("KERNEL_TRACE", "0")))
    if trace:
        _install_ntff_hook()
    tmpdir = os.environ.get("KERNEL_TRACE_DIR") or None
    r = run_bass_kernel_spmd(
        nc, in_maps, list(range(N_CORES)), trace=trace, tmpdir=tmpdir
    )
    LAST_EXEC_TIME_NS = r.exec_time_ns

    total = sum(float(r.results[i]["out"][0, 0]) for i in range(N_CORES))
    return np.float32(total / N)



# revision 5
# speedup vs baseline: 1.7019x; 1.7019x over previous
"""Distributed contrastive loss kernel for 8 Trainium2 NeuronCores.

loss = mean_i( logsumexp_j(f1n_i . f2n_j / T) - (f1n_i . f2n_i) / T )
with f1n/f2n the L2-row-normalized feature matrices, N=16384, D=512.

v2 design (fp8 DoubleRow):
- f1 rows sharded 8 ways (2048/core); f2 rows likewise.
- f2 path first: per 512-row chunk, normalize rows on DVE (inv2*32 folded into
  an fp8 cast), PE-transpose to [512, 512] fp8 blocks, AllGather the chunk.
  Four chunked AllGathers pipeline with the rest of prep + main loop.
- f1 path: raw rows cast to fp8, PE-transposed into a [128, 4, 2048] weight
  tile (k-subtile layout for DoubleRow). inv1/T/32 folded into the Exp scale.
- Main loop: per (chunk g, rank-half h, m-tile) unit, 8 DoubleRow fp8 matmuls
  (2 k-pairs x 4 ranks) accumulate a [128, 2048] f32 logits tile in PSUM,
  then ONE ScalarE Exp with per-partition scale = inv1/(T*32) and fused
  accum_out produces row-wise exp-sums. Logits never leave PSUM.
- diag from an elementwise f32 dot of the local shards; logsumexp via Ln of
  accumulated row sums; per-core partial reduced with a ones-matmul. Host
  sums 8 partials / N.
"""

import os
from contextlib import ExitStack
from functools import lru_cache

import numpy as np

import concourse.bass as bass
import concourse.mybir as mybir
import concourse.tile as tile
from concourse.bass_utils import run_bass_kernel_spmd
from concourse.masks import make_identity

# Problem shape (hardcoded per contest rules).
N = 16384
D = 512
N_CORES = 8
M_LOCAL = N // N_CORES  # 2048 rows per core
TEMP = 0.07

P = 128                 # SBUF partitions
DC = D // P             # 4 contraction chunks of 128
MT = M_LOCAL // P       # 16 m-tiles (and f2-shard tiles) per core
GW = 512                # AllGather chunk width (f2 rows per chunk)
NG = M_LOCAL // GW      # 4 AllGather chunks
GT = GW // P            # 4 f2-shard tiles per chunk
EXPW = 2048             # exp tile width = 4 ranks x 512 cols
F2S = 32.0              # power-of-2 scale folded into f2n fp8 cast
F32 = mybir.dt.float32
BF16 = mybir.dt.bfloat16
F8 = mybir.dt.float8e4
AF = mybir.ActivationFunctionType
ALU = mybir.AluOpType
DR = mybir.MatmulPerfMode.DoubleRow

# Module-level stash for the last run's profile (read by test.py).
LAST_EXEC_TIME_NS = None


def _install_ntff_hook():
    """Provide antenv.axon_hooks (missing from this image) so that
    run_bass_kernel_spmd(trace=True) can capture NTFF profiles via the
    axon PJRT .so. Mirrors trn_agent_boot.trn_boot._ntff_profile_via_ctypes."""
    import contextlib
    import ctypes
    import sys
    import types

    try:
        import antenv.axon_hooks  # noqa: F401

        return
    except ImportError:
        pass

    so_path = "/opt/axon/libaxon_pjrt.so"
    hook = None
    try:
        lib = ctypes.CDLL(so_path)
        if hasattr(lib, "axon_start_nrt_profile"):
            lib.axon_start_nrt_profile.argtypes = [
                ctypes.POINTER(ctypes.c_int64),
                ctypes.c_size_t,
            ]
            lib.axon_start_nrt_profile.restype = ctypes.c_int64
            lib.axon_stop_nrt_profile.argtypes = [ctypes.c_char_p]
            lib.axon_stop_nrt_profile.restype = ctypes.c_int64

            @contextlib.contextmanager
            def _hook(output_dir, device_ids):
                import jax

                jax.devices()
                if device_ids:
                    ids = (ctypes.c_int64 * len(device_ids))(*device_ids)
                    rc = lib.axon_start_nrt_profile(ids, len(device_ids))
                else:
                    rc = lib.axon_start_nrt_profile(None, 0)
                if rc != 0:
                    raise RuntimeError(f"axon_start_nrt_profile rc={rc}")
                try:
                    yield
                finally:
                    n = lib.axon_stop_nrt_profile(str(output_dir).encode())
                    print(f"profile: {n} file(s) written to {output_dir}", file=sys.stderr)

            hook = _hook
    except OSError:
        pass

    import antenv

    mod = types.ModuleType("antenv.axon_hooks")
    _state = {"hook": hook}
    mod.set_axon_ntff_profile_hook = lambda h: _state.__setitem__("hook", h)
    mod.get_axon_ntff_profile_hook = lambda: _state["hook"]
    sys.modules["antenv.axon_hooks"] = mod
    antenv.axon_hooks = mod

    # Artifact upload needs external storage creds; make it a no-op.
    import concourse.bass_utils as _bu

    _bu.upload_artifacts = lambda tmpdir: f"local:{tmpdir}"


def _build_bass():
    nc = bass.Bass(num_devices=N_CORES, debug=False)

    f1s = nc.dram_tensor("f1s", [M_LOCAL, D], F32, kind="ExternalInput")
    f2o = nc.dram_tensor("f2o", [M_LOCAL, D], F32, kind="ExternalInput")
    out = nc.dram_tensor("out", [1, 1], F32, kind="ExternalOutput")

    inv_temp = 1.0 / TEMP

    with tile.TileContext(nc) as tc, ExitStack() as ctx:
        consts = ctx.enter_context(tc.tile_pool(name="consts", bufs=1))
        resident = ctx.enter_context(tc.tile_pool(name="resident", bufs=1))
        work = ctx.enter_context(tc.tile_pool(name="work", bufs=3))
        stats = ctx.enter_context(tc.tile_pool(name="stats", bufs=4))
        rhsp = ctx.enter_context(tc.tile_pool(name="rhsp", bufs=2))
        psum = ctx.enter_context(tc.tile_pool(name="psum", bufs=2, space="PSUM"))
        dram = ctx.enter_context(tc.tile_pool(name="dram", bufs=1, space="DRAM"))

        identity = consts.tile([P, P], BF16)
        make_identity(nc, identity)
        ones_col = consts.tile([P, 1], F32)
        nc.vector.memset(ones_col, 1.0)

        # Residents.
        f1p = resident.tile([P, DC, M_LOCAL], F8, name="f1p")
        f2T = [resident.tile([P, DC, GW], F8, name=f"f2T{g}") for g in range(NG)]
        x1res = [resident.tile([P, D], F32, name=f"x1r{t}") for t in range(MT)]
        x2res = [resident.tile([P, D], F32, name=f"x2r{t}") for t in range(MT)]
        ss1g = resident.tile([P, MT], F32, name="ss1g")
        ss2g = resident.tile([P, MT], F32, name="ss2g")
        diag_raw = resident.tile([P, MT], F32, name="diag_raw")
        diag = resident.tile([P, MT], F32, name="diag")
        inv1g = resident.tile([P, MT], F32, name="inv1g")
        inv2g = resident.tile([P, MT], F32, name="inv2g")
        i232 = resident.tile([P, MT], F32, name="i232")
        scale1g = resident.tile([P, MT], F32, name="scale1g")
        rowsums = [
            resident.tile([P, NG * 2], F32, name=f"rs{mt}") for mt in range(MT)
        ]
        losses = resident.tile([P, MT], F32, name="losses")

        # AllGather bounce buffers, one pair per chunk.
        agin = [dram.tile([DC * P, GW], F8, name=f"agin{g}") for g in range(NG)]
        agout = [
            dram.tile([N_CORES * DC * P, GW], F8, name=f"agout{g}",
                      addr_space="Shared")
            for g in range(NG)
        ]

        def sumsq_col(x, col, tag):
            """sum(x^2) per row of [P, D] f32 tile -> col ([P,1] slice). DVE only."""
            st = stats.tile([P, nc.vector.BN_STATS_DIM], F32, tag="bst", name=f"st{tag}")
            nc.vector.bn_stats(st, x)
            mv = stats.tile([P, nc.vector.BN_AGGR_DIM], F32, tag="bmv", name=f"mv{tag}")
            nc.vector.bn_aggr(mv, st)
            m2 = stats.tile([P, 1], F32, tag="m2", name=f"m2{tag}")
            nc.vector.tensor_tensor(m2, mv[:, 0:1], mv[:, 0:1], ALU.mult)
            s2 = stats.tile([P, 1], F32, tag="s2", name=f"s2{tag}")
            nc.vector.tensor_tensor(s2, mv[:, 1:2], m2, ALU.add)
            nc.vector.tensor_scalar_mul(col, s2, float(D))

        def rsqrt_quarter(dst, src, gsl, tag):
            """dst[:, gsl] = 1/sqrt(src[:, gsl]) via exp(-0.5*ln(.)) - one table set."""
            ln = stats.tile([P, GT], F32, tag=f"ln{tag}", name=f"ln{tag}")
            nc.scalar.activation(ln, src[:, gsl], AF.Ln)
            nc.scalar.activation(dst[:, gsl], ln, AF.Exp, scale=-0.5)

        # ---- Phase F2: normalize + transpose + AllGather, chunk by chunk ----
        # All x2 loads first so DMA engines run ahead freely.
        for t in range(MT):
            nc.sync.dma_start(out=x2res[t], in_=f2o[t * P : (t + 1) * P, :])

        def f2_group(g):
            gsl = slice(g * GT, (g + 1) * GT)
            for t in range(g * GT, (g + 1) * GT):
                sumsq_col(x2res[t], ss2g[:, t : t + 1], "2")
            rsqrt_quarter(inv2g, ss2g, gsl, "2")
            nc.vector.tensor_scalar_mul(i232[:, gsl], inv2g[:, gsl], F2S)
            for t in range(g * GT, (g + 1) * GT):
                tt = t - g * GT
                x2c = work.tile([P, D], BF16, tag="x2c", name="x2c")
                nc.vector.tensor_scalar_mul(x2c, x2res[t], i232[:, t : t + 1])
                for c in range(DC):
                    tp = psum.tile([P, P], BF16, tag="ps", name="tp2")
                    nc.tensor.transpose(tp, x2c[:, c * P : (c + 1) * P], identity)
                    nc.vector.tensor_copy(
                        f2T[g][:, c, tt * P : (tt + 1) * P], tp
                    )
            for c in range(DC):
                nc.sync.dma_start(
                    out=agin[g][c * P : (c + 1) * P, :], in_=f2T[g][:, c, :]
                )
            nc.gpsimd.collective_compute(
                "AllGather",
                ALU.bypass,
                replica_groups=[list(range(N_CORES))],
                ins=[agin[g][:, :].opt()],
                outs=[agout[g][:, :].opt()],
            )

        for g in range(NG):
            f2_group(g)

        # ---- Phase F1: load, stats, diag, cast, transpose -------------------
        for t in range(MT):
            nc.sync.dma_start(out=x1res[t], in_=f1s[t * P : (t + 1) * P, :])

        def f1_group(g):
            gsl = slice(g * GT, (g + 1) * GT)
            for t in range(g * GT, (g + 1) * GT):
                sumsq_col(x1res[t], ss1g[:, t : t + 1], "1")
                prod = work.tile([P, D], F32, tag="prod", bufs=2, name="prod")
                nc.vector.tensor_tensor(prod, x1res[t], x2res[t], ALU.mult)
                nc.vector.reduce_sum(
                    diag_raw[:, t : t + 1], prod, axis=mybir.AxisListType.X
                )
                x1c = work.tile([P, D], BF16, tag="x1c", name="x1c")
                nc.vector.tensor_copy(x1c, x1res[t])
                for c in range(DC):
                    tp = psum.tile([P, P], BF16, tag="ps", name="tp1")
                    nc.tensor.transpose(tp, x1c[:, c * P : (c + 1) * P], identity)
                    nc.vector.tensor_copy(
                        f1p[:, c, t * P : (t + 1) * P], tp
                    )
            rsqrt_quarter(inv1g, ss1g, gsl, "1")
            nc.vector.tensor_scalar_mul(
                scale1g[:, gsl], inv1g[:, gsl], inv_temp / F2S
            )
            # diag = diag_raw * inv1 * inv2 (row-paired).
            nc.vector.tensor_tensor(
                diag[:, gsl], diag_raw[:, gsl], inv1g[:, gsl], ALU.mult
            )
            nc.vector.tensor_tensor(
                diag[:, gsl], diag[:, gsl], inv2g[:, gsl], ALU.mult
            )

        for g in range(NG):
            f1_group(g)

        # ---- Phase 3: fused logits -> exp(scale=inv1/(T*32)) -> row-sums ----
        def main_group(g):
            for h in range(2):
                rh = rhsp.tile([P, DC, EXPW], F8, tag="rh", name="rh")
                for j in range(4):
                    r = 4 * h + j
                    for c in range(DC):
                        nc.sync.dma_start(
                            out=rh[:, c, j * GW : (j + 1) * GW],
                            in_=agout[g][r * D + c * P : r * D + (c + 1) * P, :],
                        )
                for mt in range(MT):
                    ps = psum.tile([P, EXPW], F32, tag="ps", name="ps")
                    for cp in range(2):
                        lhsT = f1p[:, 2 * cp : 2 * cp + 2, mt * P : (mt + 1) * P]
                        for j in range(4):
                            nc.tensor.matmul(
                                ps[:, j * GW : (j + 1) * GW],
                                lhsT=lhsT,
                                rhs=rh[:, 2 * cp : 2 * cp + 2, j * GW : (j + 1) * GW],
                                start=(cp == 0),
                                stop=(cp == 1),
                                perf_mode=DR,
                            )
                    ex = work.tile([P, EXPW], BF16, tag="ex", bufs=2, name="ex")
                    nc.scalar.activation(
                        ex,
                        ps,
                        AF.Exp,
                        scale=scale1g[:, mt : mt + 1],
                        accum_out=rowsums[mt][:, 2 * g + h : 2 * g + h + 1],
                    )

        for g in range(NG):
            main_group(g)

        # ---- Phase 4: logsumexp, subtract diag, reduce -----------------------
        for mt in range(MT):
            s = stats.tile([P, 1], F32, tag="s", name="s")
            nc.vector.reduce_sum(s, rowsums[mt], axis=mybir.AxisListType.X)
            lse = stats.tile([P, 1], F32, tag="lse", name="lse")
            nc.scalar.activation(lse, s, AF.Ln)
            # losses[:, mt] = lse - diag/T = (diag * -1/T) + lse
            nc.vector.scalar_tensor_tensor(
                out=losses[:, mt : mt + 1],
                in0=diag[:, mt : mt + 1],
                scalar=-inv_temp,
                in1=lse,
                op0=ALU.mult,
                op1=ALU.add,
            )

        loss_col = stats.tile([P, 1], F32, tag="lc", name="loss_col")
        nc.vector.reduce_sum(loss_col, losses, axis=mybir.AxisListType.X)
        fin = psum.tile([1, 1], F32, tag="ps", name="fin")
        nc.tensor.matmul(fin, lhsT=loss_col, rhs=ones_col, start=True, stop=True)
        res = stats.tile([1, 1], F32, tag="res", name="res")
        nc.any.tensor_copy(res, fin)
        nc.sync.dma_start(out=out[:, :], in_=res)

    return nc


_WAIT_EXEMPT = ("InstCall",)


def _legalize_sync_waits(nc, limit=1):
    """Walrus codegen rejects instructions carrying more than ~1 embedded
    semaphore wait ("Too many sync wait commands"). Move excess waits onto
    injected same-engine NoOps (one wait each) ahead of the instruction —
    semantically identical (the engine blocks on the NoOps first)."""
    n_split = 0
    for b in nc.m.functions[0].blocks:
        insts = b.instructions
        out = []
        changed = False
        for ins in insts:
            si = ins.sync_info
            tname = type(ins).__name__
            if (
                si is not None
                and len(si.on_wait) > limit
                and tname not in _WAIT_EXEMPT
            ):
                waits = list(si.on_wait)
                keep, excess = waits[:limit], waits[limit:]
                for j, w in enumerate(excess):
                    noop = mybir.InstNoOp(name=f"{ins.name}-ws{j}", ins=[], outs=[])
                    noop.engine = ins.engine
                    noop.sync_info = mybir.SyncInfo(on_wait=[w], on_update=[])
                    out.append(noop)
                ins.sync_info = mybir.SyncInfo(
                    on_wait=keep, on_update=list(si.on_update)
                )
                n_split += 1
                changed = True
            out.append(ins)
        if changed:
            b.instructions = out
    return n_split


@lru_cache(maxsize=1)
def _get_nc():
    nc = _build_bass()
    _legalize_sync_waits(nc)
    return nc


def kernel(features1, features2):
    global LAST_EXEC_TIME_NS
    f1 = np.ascontiguousarray(np.asarray(features1, dtype=np.float32))
    f2 = np.ascontiguousarray(np.asarray(features2, dtype=np.float32))
    assert f1.shape == (N, D) and f2.shape == (N, D)

    in_maps = []
    for i in range(N_CORES):
        sl = slice(i * M_LOCAL, (i + 1) * M_LOCAL)
        in_maps.append(
            {
                "f1s": np.ascontiguousarray(f1[sl]),
                "f2o": np.ascontiguousarray(f2[sl]),
            }
        )

    nc = _get_nc()
    trace = bool(int(os.environ.get("KERNEL_TRACE", "0")))
    if trace:
        _install_ntff_hook()
    tmpdir = os.environ.get("KERNEL_TRACE_DIR") or None
    r = run_bass_kernel_spmd(
        nc, in_maps, list(range(N_CORES)), trace=trace, tmpdir=tmpdir
    )
    LAST_EXEC_TIME_NS = r.exec_time_ns

    total = sum(float(r.results[i]["out"][0, 0]) for i in range(N_CORES))
    return np.float32(total / N)
